# revision 3
# baseline (speedup 1.0000x reference)
"""Chunked bf16 kernel with ncfw AllGathers, asymmetric chunks (48/16).

r2 changes vs the 392us r1+ kernel:
- last-chunk j-split loads come from a dedicated 16-buf pool so their DMA
  issue is not gated on reduce completions (was an 8us stream gap + drain)
- asymmetric chunks: the final AllGather covers only 16 batches, so the
  pre-AG tail compute and the AG payload shrink
- chunk-1 glc copies moved to DVE (ACT chain shortened), chunk-0 output
  columns stored to DRAM during the final AG wait, sync-ring ordering set
  so the early store is not queued behind the AG-dependent gather load."""

import sys

if "/opt/trn_rl_repo" not in sys.path:
    sys.path.insert(0, "/opt/trn_rl_repo")

import numpy as np

B_FULL = 512
C_IN = 2048
T_POOL = 196
O_OUT = 512
N_CORES = 8

CK0 = 48          # batches in chunk 0 (AG fully overlapped by the stream)
CK1 = 16          # batches in chunk 1 (the exposed tail chunk)


def build_kernel(b_full, c_in, t_pool, o_out, n_cores, ft_bufs=6):
    import concourse.mybir as mybir
    import concourse.tile as tile
    from concourse import bacc
    from concourse.masks import make_identity

    f32 = mybir.dt.float32
    bf16 = mybir.dt.bfloat16
    AL = mybir.AluOpType
    AF = mybir.ActivationFunctionType
    X = mybir.AxisListType.X

    bc = b_full // n_cores
    nj = 16
    cks = [CK0, CK1]
    oc = o_out // 128
    nr = n_cores
    assert c_in == 128 * nj and CK0 + CK1 == bc and o_out % 128 == 0

    nc = bacc.Bacc("TRN2", target_bir_lowering=False, debug=False,
                   enable_asserts=False, num_devices=n_cores)
    feat = nc.dram_tensor("features", [bc, c_in, t_pool], f32,
                          kind="ExternalInput").ap()
    w_in = nc.dram_tensor("w", [o_out, c_in], f32, kind="ExternalInput").ap()
    bias_in = nc.dram_tensor("bias", [1, o_out], f32, kind="ExternalInput").ap()
    out_d = nc.dram_tensor("out", [bc, b_full], f32, kind="ExternalOutput").ap()

    with tile.TileContext(nc) as tc:
        with (
            tc.tile_pool(name="const", bufs=1) as constp,
            tc.tile_pool(name="wload", bufs=1) as wlp,
            tc.tile_pool(name="wtp", bufs=1) as wtp,
            tc.tile_pool(name="featp", bufs=ft_bufs) as fp,
            tc.tile_pool(name="featl", bufs=16) as flp,
            tc.tile_pool(name="poolp", bufs=1) as lp,
            tc.tile_pool(name="postp", bufs=1) as pp,
            tc.tile_pool(name="psrot", bufs=2, space="PSUM") as psp,
            tc.tile_pool(name="psgps", bufs=2, space="PSUM") as psgp,
            tc.tile_pool(name="pssim", bufs=1, space="PSUM") as pssp,
            tc.tile_pool(name="pssimh", bufs=1, space="PSUM") as pssp2,
            tc.tile_pool(name="dram", bufs=1, space="DRAM") as dp,
        ):
            # ---- constants ----
            identf = constp.tile([128, 128], f32, name="identf")
            make_identity(nc, identf)
            identb = constp.tile([CK0, CK0], bf16, name="identb")
            make_identity(nc, identb)
            ones = constp.tile([1, CK0], bf16, name="ones")
            nc.vector.memset(ones, 1.0)
            bias_sb = constp.tile([1, o_out], f32, name="bias_sb")
            nc.sync.dma_start(bias_sb[:], bias_in[:])
            bias_t = constp.tile([1, o_out], bf16, name="bias_t")
            nc.scalar.mul(bias_t[:], bias_sb[:], float(t_pool))

            # ---- W^T in bf16 ----
            wl = []
            for l in range(oc):
                wli = wlp.tile([128, c_in], f32, name=f"wl{l}")
                # balance the 4MB of W across both rings so neither ring
                # carries extra bytes and the feature stream tails end together
                eng = nc.sync if l % 2 == 0 else nc.scalar
                eng.dma_start(wli[:], w_in[l * 128:(l + 1) * 128, :])
                wl.append(wli)
            wt = []
            for j in range(nj):
                pswt = psp.tile([128, o_out], f32, name="pswt", tag="rot")
                for l in range(oc):
                    src = wl[l][:, :].rearrange("o (p j) -> o p j", j=nj)[:, :, j]
                    nc.tensor.transpose(pswt[:, l * 128:(l + 1) * 128],
                                        src, identf[:])
                wtj = wtp.tile([128, o_out], bf16, name=f"wt{j}")
                nc.scalar.copy(wtj[:], pswt[:])
                wt.append(wtj)

            gl_full = pp.tile([128, oc, bc], bf16, name="gl_full")
            outsb = pp.tile([bc, b_full], f32, name="outsb")
            glcs = [pp.tile([128, oc * cks[c]], bf16, name=f"glc{c}")
                    for c in range(2)]
            grts = []

            JSPLIT = 4  # trailing batches of the last chunk streamed j-major

            def pool_chunk(c):
                off = 0 if c == 0 else CK0
                ck = cks[c]
                p4 = lp.tile([128, ck, nj], bf16, name=f"p4_{c}")
                split = JSPLIT if c == 1 else 0
                for i in range(ck - split):
                    b = off + i
                    ft = fp.tile([128, nj * t_pool], f32, name="ft")
                    src = feat[b:b + 1, :, :].rearrange(
                        "b (p j) t -> p (b j t)", j=nj)
                    dma_eng = nc.scalar if b % 2 == 0 else nc.sync
                    dma_eng.dma_start(ft[:], src)
                    with nc.allow_low_precision("pooled sums cast to bf16"):
                        nc.vector.reduce_sum(
                            p4[:, i, :],
                            ft[:].rearrange("p (j t) -> p j t", t=t_pool),
                            axis=X)
                # Stream the last `split` batches j-group-major from a
                # dedicated 16-buf pool: all 16 DMAs issue immediately (no
                # reduce-gating), so the last bytes follow the main stream
                # with no ring gap. Once group g of every batch has landed,
                # the projection matmuls j = 4g..4g+3 run overlapped with
                # the remaining groups' streaming.
                n = 0
                for g in range(4):
                    for i in range(ck - split, ck):
                        b = off + i
                        ftj = flp.tile([128, 4, t_pool], f32, name="ftl")
                        eng = nc.scalar if n % 2 == 0 else nc.sync
                        n += 1
                        src = feat[b:b + 1, :, :].rearrange(
                            "b (p j) t -> p (b j) t", j=nj)
                        eng.dma_start(ftj[:], src[:, 4 * g:4 * (g + 1), :])
                        with nc.allow_low_precision("pooled bf16"):
                            nc.vector.reduce_sum(
                                p4[:, i, 4 * g:4 * (g + 1)], ftj[:], axis=X)
                return p4

            def project(c, p4):
                ck = cks[c]
                gps = psgp.tile([ck, o_out], f32, name="gps", tag="gps")
                for j in range(nj):
                    nc.tensor.matmul(gps[:], p4[:, :, j], wt[j][:],
                                     start=(j == 0), stop=False)
                nc.tensor.matmul(gps[:], ones[:, :ck], bias_t[:],
                                 start=False, stop=True)
                return gps

            out_rv = out_d[:, :].rearrange("b (r i) -> b r i", i=bc)
            osb_rv = outsb[:, :].rearrange("b (r i) -> b r i", i=bc)

            # ================= chunk 0 =================
            p4 = pool_chunk(0)
            gps = project(0, p4)
            scr = pp.tile([CK0, o_out], f32, name="scr0")
            n2 = pp.tile([CK0, 1], f32, name="n20")
            nc.scalar.activation(scr[:], gps[:], AF.Square, accum_out=n2[:])
            gsb = pp.tile([CK0, o_out], f32, name="gsb0")
            nc.scalar.copy(gsb[:], gps[:])
            nrm = pp.tile([CK0, 1], f32, name="nrm0")
            nc.scalar.sqrt(nrm[:], n2[:])
            gn0 = pp.tile([CK0, o_out], bf16, name="gn0")
            nc.gpsimd.normalize_recip(gn0[:], gsb[:], nrm[:])
            glc_v0 = glcs[0][:].rearrange("p (m i) -> p m i", i=CK0)
            for m in range(oc):
                psg = psp.tile([128, CK0], bf16, name="psg", tag="rot")
                nc.tensor.transpose(psg[:], gn0[:, m * 128:(m + 1) * 128],
                                    identb[:])
                nc.scalar.copy(gl_full[:, m, 0:CK0], psg[:])
                nc.scalar.copy(glc_v0[:, m, :], psg[:])
            # AG0: overlapped by the chunk-1 stream; all adjacent DMAs SWDGE
            agin0 = dp.tile([128, oc * CK0], bf16, name="agin0")
            agout0 = dp.tile([nr * 128, oc * CK0], bf16, name="agout0",
                             addr_space="Shared")
            nc.gpsimd.dma_start(agin0[:], glcs[0][:])
            nc.gpsimd.collective_compute(
                "AllGather", AL.bypass,
                replica_groups=[list(range(n_cores))],
                ins=[agin0.opt()], outs=[agout0.opt()],
            )
            grt0 = pp.tile([128, nr, oc * CK0], bf16, name="grt0")
            nc.gpsimd.dma_start(
                grt0[:], agout0[:, :].rearrange("(r p) f -> p r f", r=nr))

            # ================= chunk 1 =================
            p4 = pool_chunk(1)
            gps = project(1, p4)
            scr1 = pp.tile([CK1, o_out], f32, name="scr1")
            n21 = pp.tile([CK1, 1], f32, name="n21")
            nc.scalar.activation(scr1[:], gps[:], AF.Square, accum_out=n21[:])
            nrm1 = pp.tile([CK1, 1], f32, name="nrm1")
            nc.scalar.sqrt(nrm1[:], n21[:])
            rinv1 = pp.tile([CK1, 1], f32, name="rinv1")
            nc.vector.reciprocal(rinv1[:], nrm1[:])
            gn1 = pp.tile([CK1, o_out], bf16, name="gn1")
            glc_v1 = glcs[1][:].rearrange("p (m i) -> p m i", i=CK1)
            # scale per 128-block so each transpose starts as soon as its
            # block is written; glc copies on DVE keep the ACT chain short
            for m in range(oc):
                nc.scalar.mul(gn1[:, m * 128:(m + 1) * 128],
                              gps[:, m * 128:(m + 1) * 128], rinv1[:])
                psg = psp.tile([128, CK1], bf16, name="psg1", tag="rot")
                nc.tensor.transpose(psg[:], gn1[:, m * 128:(m + 1) * 128],
                                    identb[:CK1, :CK1])
                nc.vector.tensor_copy(glc_v1[:, m, :], psg[:])
                nc.scalar.copy(gl_full[:, m, CK0:bc], psg[:])
            # AG1 trigger ASAP (sync ring is idle once the stream is done)
            agin1 = dp.tile([128, oc * CK1], bf16, name="agin1")
            agout1 = dp.tile([nr * 128, oc * CK1], bf16, name="agout1",
                             addr_space="Shared")
            nc.sync.dma_start(agin1[:], glcs[1][:])
            nc.gpsimd.collective_compute(
                "AllGather", AL.bypass,
                replica_groups=[list(range(n_cores))],
                ins=[agin1.opt()], outs=[agout1.opt()],
            )

            # chunk 0 sim runs during the AG1 wait: one [bc, 8*CK0] block,
            # then its output columns stored to DRAM (issued on the sync
            # ring BEFORE the AG1-dependent gather load so it isn't queued
            # behind the collective).
            simps = pssp.tile([bc, nr * CK0], f32, name="simps0", tag="sim")
            for m in range(oc):
                nc.tensor.matmul(
                    simps[:], gl_full[:, m, :],
                    grt0[:, :, m * CK0:(m + 1) * CK0],
                    start=(m == 0), stop=(m == oc - 1))
            dst0 = osb_rv[:, :, 0:CK0]
            nc.vector.tensor_copy(dst0, simps[:])
            nc.sync.dma_start(out_rv[:, :, 0:CK0], dst0)

            # gather load split across both idle HWDGE rings; chunk-1 sim
            # per rank-half so each half's matmuls start as soon as its
            # half of the split load lands
            grt1 = pp.tile([128, nr, oc * CK1], bf16, name="grt1")
            agv1 = agout1[:, :].rearrange("(r p) f -> p r f", r=nr)
            hr = nr // 2
            nc.sync.dma_start(grt1[:, :hr, :], agv1[:, :hr, :])
            nc.scalar.dma_start(grt1[:, hr:, :], agv1[:, hr:, :])
            for h in range(2):
                sph = pssp2.tile([bc, hr * CK1], f32, name=f"simh{h}",
                                 tag=f"simh{h}")
                for m in range(oc):
                    nc.tensor.matmul(
                        sph[:], gl_full[:, m, :],
                        grt1[:, h * hr:(h + 1) * hr,
                             m * CK1:(m + 1) * CK1],
                        start=(m == 0), stop=(m == oc - 1))
                dsth = osb_rv[:, h * hr:(h + 1) * hr, CK0:bc]
                nc.vector.tensor_copy(dsth, sph[:])
                eng = nc.sync if h == 0 else nc.scalar
                eng.dma_start(out_rv[:, h * hr:(h + 1) * hr, CK0:bc], dsth)

    nc.compile()
    return nc


_NC_CACHE = {}


def _get_nc():
    key = (B_FULL, C_IN, T_POOL, O_OUT, N_CORES)
    if key not in _NC_CACHE:
        _NC_CACHE[key] = build_kernel(*key)
    return _NC_CACHE[key]


def _run(features, W, bias, trace=False, tmpdir=None):
    from concourse.bass_utils import run_bass_kernel_spmd

    feats = np.ascontiguousarray(np.asarray(features, dtype=np.float32))
    w_np = np.ascontiguousarray(np.asarray(W, dtype=np.float32))
    bias_np = np.ascontiguousarray(
        np.asarray(bias, dtype=np.float32).reshape(1, O_OUT))
    bc = B_FULL // N_CORES

    nc = _get_nc()
    in_maps = [
        {"features": feats[r * bc:(r + 1) * bc], "w": w_np, "bias": bias_np}
        for r in range(N_CORES)
    ]
    kw = {"tmpdir": tmpdir} if tmpdir else {}
    res = run_bass_kernel_spmd(nc, in_maps, core_ids=list(range(N_CORES)),
                               trace=trace, **kw)
    out = np.concatenate([res.results[r]["out"] for r in range(N_CORES)], axis=0)
    return out, res.exec_time_ns


def kernel(features, W, bias):
    out, _ = _run(features, W, bias)
    return out


# revision 4
# speedup vs baseline: 1.0925x; 1.0925x over previous
"""Chunked bf16 kernel with per-chunk ncfw AllGathers (4 x 16 batches).

r3 design (evidence from the r1/r2 traces):
- the feature stream runs at the HBM cap (~357 GB/s) when nothing gates
  it: per-batch DMAs alternate the two HWDGE rings, the last 4 batches
  stream j-group-major from a dedicated 16-buf pool (no reduce-gating)
- ncfw mesh AllGather cost grows steeply with payload (16KB -> ~24us,
  32KB -> ~30-46us, 48KB -> ~97us), so use four 16KB AGs: the first
  three hide under the stream, only the last one (~24us) is exposed
- mid-chunk collective-adjacent work stays off the stream rings and off
  DVE: norm via ACT square/sqrt + gpsimd normalize, copies on ACT,
  agin/grt DMAs on gpsimd (SWDGE)
- tail: last chunk normalizes via ACT muls + DVE copies for the fastest
  possible trigger, all sim matmuls run during the final AG wait, one
  row-split output store at the end."""

import sys

if "/opt/trn_rl_repo" not in sys.path:
    sys.path.insert(0, "/opt/trn_rl_repo")

import numpy as np

B_FULL = 512
C_IN = 2048
T_POOL = 196
O_OUT = 512
N_CORES = 8

CKS = [16, 16, 16, 16]   # batches per chunk; one AllGather per chunk
JSPLIT = 4               # trailing batches of the LAST chunk streamed j-major


def build_kernel(b_full, c_in, t_pool, o_out, n_cores, ft_bufs=6):
    import concourse.mybir as mybir
    import concourse.tile as tile
    from concourse import bacc
    from concourse.masks import make_identity

    f32 = mybir.dt.float32
    bf16 = mybir.dt.bfloat16
    AL = mybir.AluOpType
    AF = mybir.ActivationFunctionType
    X = mybir.AxisListType.X

    bc = b_full // n_cores
    nj = 16
    cks = list(CKS)
    nch = len(cks)
    offs = [sum(cks[:c]) for c in range(nch)]
    ckmax = max(cks)
    oc = o_out // 128
    nr = n_cores
    assert c_in == 128 * nj and sum(cks) == bc and o_out % 128 == 0

    nc = bacc.Bacc("TRN2", target_bir_lowering=False, debug=False,
                   enable_asserts=False, num_devices=n_cores)
    feat = nc.dram_tensor("features", [bc, c_in, t_pool], f32,
                          kind="ExternalInput").ap()
    w_in = nc.dram_tensor("w", [o_out, c_in], f32, kind="ExternalInput").ap()
    bias_in = nc.dram_tensor("bias", [1, o_out], f32, kind="ExternalInput").ap()
    out_d = nc.dram_tensor("out", [bc, b_full], f32, kind="ExternalOutput").ap()

    with tile.TileContext(nc) as tc:
        with (
            tc.tile_pool(name="const", bufs=1) as constp,
            tc.tile_pool(name="wload", bufs=1) as wlp,
            tc.tile_pool(name="wtp", bufs=1) as wtp,
            tc.tile_pool(name="featp", bufs=ft_bufs) as fp,
            tc.tile_pool(name="featl", bufs=16) as flp,
            tc.tile_pool(name="poolp", bufs=1) as lp,
            tc.tile_pool(name="normp", bufs=2) as np_,
            tc.tile_pool(name="postp", bufs=1) as pp,
            tc.tile_pool(name="psrot", bufs=2, space="PSUM") as psp,
            tc.tile_pool(name="psgps", bufs=2, space="PSUM") as psgp,
            tc.tile_pool(name="pssim", bufs=2, space="PSUM") as pssp,
            tc.tile_pool(name="dram", bufs=1, space="DRAM") as dp,
        ):
            # ---- constants ----
            identf = constp.tile([128, 128], f32, name="identf")
            make_identity(nc, identf)
            identb = constp.tile([ckmax, ckmax], bf16, name="identb")
            make_identity(nc, identb)
            ones = constp.tile([1, ckmax], bf16, name="ones")
            nc.vector.memset(ones, 1.0)
            bias_sb = constp.tile([1, o_out], f32, name="bias_sb")
            nc.sync.dma_start(bias_sb[:], bias_in[:])
            bias_t = constp.tile([1, o_out], bf16, name="bias_t")
            nc.scalar.mul(bias_t[:], bias_sb[:], float(t_pool))

            # ---- W^T in bf16 ----
            wl = []
            for l in range(oc):
                wli = wlp.tile([128, c_in], f32, name=f"wl{l}")
                eng = nc.sync if l % 2 == 0 else nc.scalar
                eng.dma_start(wli[:], w_in[l * 128:(l + 1) * 128, :])
                wl.append(wli)
            wt = []
            for j in range(nj):
                pswt = psp.tile([128, o_out], f32, name="pswt", tag="rot")
                for l in range(oc):
                    src = wl[l][:, :].rearrange("o (p j) -> o p j", j=nj)[:, :, j]
                    nc.tensor.transpose(pswt[:, l * 128:(l + 1) * 128],
                                        src, identf[:])
                wtj = wtp.tile([128, o_out], bf16, name=f"wt{j}")
                nc.scalar.copy(wtj[:], pswt[:])
                wt.append(wtj)

            gl_full = pp.tile([128, oc, bc], bf16, name="gl_full")
            outsb = pp.tile([bc, b_full], f32, name="outsb")
            glcs = [pp.tile([128, oc * cks[c]], bf16, name=f"glc{c}")
                    for c in range(nch)]
            agouts = []
            grts = []

            def pool_chunk(c):
                off, ck = offs[c], cks[c]
                p4 = lp.tile([128, ck, nj], bf16, name=f"p4_{c}")
                split = JSPLIT if c == nch - 1 else 0
                for i in range(ck - split):
                    b = off + i
                    ft = fp.tile([128, nj * t_pool], f32, name="ft")
                    src = feat[b:b + 1, :, :].rearrange(
                        "b (p j) t -> p (b j t)", j=nj)
                    dma_eng = nc.scalar if b % 2 == 0 else nc.sync
                    dma_eng.dma_start(ft[:], src)
                    with nc.allow_low_precision("pooled sums cast to bf16"):
                        nc.vector.reduce_sum(
                            p4[:, i, :],
                            ft[:].rearrange("p (j t) -> p j t", t=t_pool),
                            axis=X)
                # last chunk: stream the trailing batches j-group-major from
                # a dedicated pool (no reduce-gating) so the final group's
                # projection follows the last byte with minimal latency
                n = 0
                for g in range(4):
                    for i in range(ck - split, ck):
                        b = off + i
                        ftj = flp.tile([128, 4, t_pool], f32, name="ftl")
                        eng = nc.scalar if n % 2 == 0 else nc.sync
                        n += 1
                        src = feat[b:b + 1, :, :].rearrange(
                            "b (p j) t -> p (b j) t", j=nj)
                        eng.dma_start(ftj[:], src[:, 4 * g:4 * (g + 1), :])
                        with nc.allow_low_precision("pooled bf16"):
                            nc.vector.reduce_sum(
                                p4[:, i, 4 * g:4 * (g + 1)], ftj[:], axis=X)
                return p4

            def project(c, p4):
                ck = cks[c]
                gps = psgp.tile([ck, o_out], f32, name="gps", tag="gps")
                for j in range(nj):
                    nc.tensor.matmul(gps[:], p4[:, :, j], wt[j][:],
                                     start=(j == 0), stop=False)
                nc.tensor.matmul(gps[:], ones[:, :ck], bias_t[:],
                                 start=False, stop=True)
                return gps

            def start_ag(c, dma_eng):
                ck = cks[c]
                agin = dp.tile([128, oc * ck], bf16, name=f"agin{c}")
                agout = dp.tile([nr * 128, oc * ck], bf16, name=f"agout{c}",
                                addr_space="Shared")
                dma_eng.dma_start(agin[:], glcs[c][:])
                nc.gpsimd.collective_compute(
                    "AllGather", AL.bypass,
                    replica_groups=[list(range(n_cores))],
                    ins=[agin.opt()], outs=[agout.opt()],
                )
                agouts.append(agout)

            # ================= mid chunks =================
            for c in range(nch - 1):
                ck, off = cks[c], offs[c]
                p4 = pool_chunk(c)
                gps = project(c, p4)
                scr = np_.tile([ck, o_out], f32, name="scr")
                n2 = np_.tile([ck, 1], f32, name="n2")
                nc.scalar.activation(scr[:], gps[:], AF.Square,
                                     accum_out=n2[:])
                gsb = np_.tile([ck, o_out], f32, name="gsb")
                nc.scalar.copy(gsb[:], gps[:])
                nrm = np_.tile([ck, 1], f32, name="nrm")
                nc.scalar.sqrt(nrm[:], n2[:])
                gn = np_.tile([ck, o_out], bf16, name="gn")
                nc.gpsimd.normalize_recip(gn[:], gsb[:], nrm[:])
                glc_v = glcs[c][:].rearrange("p (m i) -> p m i", i=ck)
                for m in range(oc):
                    psg = psp.tile([128, ck], bf16, name="psg", tag="rot")
                    nc.tensor.transpose(psg[:], gn[:, m * 128:(m + 1) * 128],
                                        identb[:ck, :ck])
                    nc.scalar.copy(gl_full[:, m, off:off + ck], psg[:])
                    nc.scalar.copy(glc_v[:, m, :], psg[:])
                # collective + gather load stay on the SWDGE ring so the
                # HWDGE feature stream is never queued behind them
                start_ag(c, nc.gpsimd)
                grt = pp.tile([128, nr, oc * ck], bf16, name=f"grt{c}")
                nc.gpsimd.dma_start(
                    grt[:],
                    agouts[c][:, :].rearrange("(r p) f -> p r f", r=nr))
                grts.append(grt)

            # ================= last chunk =================
            cl = nch - 1
            ck, off = cks[cl], offs[cl]
            p4 = pool_chunk(cl)
            gps = project(cl, p4)
            scr1 = np_.tile([ck, o_out], f32, name="scr")
            n21 = np_.tile([ck, 1], f32, name="n2")
            nc.scalar.activation(scr1[:], gps[:], AF.Square, accum_out=n21[:])
            nrm1 = np_.tile([ck, 1], f32, name="nrm")
            nc.scalar.sqrt(nrm1[:], n21[:])
            rinv1 = pp.tile([ck, 1], f32, name="rinv1")
            nc.vector.reciprocal(rinv1[:], nrm1[:])
            gn1 = np_.tile([ck, o_out], bf16, name="gn")
            glc_v1 = glcs[cl][:].rearrange("p (m i) -> p m i", i=ck)
            # scale per 128-block so each transpose starts as soon as its
            # block is written; glc copies on DVE keep the ACT chain short
            for m in range(oc):
                nc.scalar.mul(gn1[:, m * 128:(m + 1) * 128],
                              gps[:, m * 128:(m + 1) * 128], rinv1[:])
                psg = psp.tile([128, ck], bf16, name="psg", tag="rot")
                nc.tensor.transpose(psg[:], gn1[:, m * 128:(m + 1) * 128],
                                    identb[:ck, :ck])
                nc.vector.tensor_copy(glc_v1[:, m, :], psg[:])
                nc.scalar.copy(gl_full[:, m, off:off + ck], psg[:])
            # trigger the final AG ASAP; sync ring is idle once the stream
            # is done
            start_ag(cl, nc.sync)

            # ---- sims for the mid chunks run during the final AG wait ----
            osb_rv = outsb[:, :].rearrange("b (r i) -> b r i", i=bc)
            for c in range(nch - 1):
                ck, off = cks[c], offs[c]
                simps = pssp.tile([bc, nr * ck], f32, name="simps",
                                  tag=f"sim{c % 2}")
                for m in range(oc):
                    nc.tensor.matmul(
                        simps[:], gl_full[:, m, :],
                        grts[c][:, :, m * ck:(m + 1) * ck],
                        start=(m == 0), stop=(m == oc - 1))
                nc.vector.tensor_copy(osb_rv[:, :, off:off + ck], simps[:])

            # ---- final chunk: split gather load, per-half sims ----
            grtl = pp.tile([128, nr, oc * ck], bf16, name=f"grt{cl}")
            agvl = agouts[cl][:, :].rearrange("(r p) f -> p r f", r=nr)
            hr = nr // 2
            nc.sync.dma_start(grtl[:, :hr, :], agvl[:, :hr, :])
            nc.scalar.dma_start(grtl[:, hr:, :], agvl[:, hr:, :])
            ck, off = cks[cl], offs[cl]
            for h in range(2):
                sph = pssp.tile([bc, hr * ck], f32, name=f"simh{h}",
                                tag=f"sim{h}")
                for m in range(oc):
                    nc.tensor.matmul(
                        sph[:], gl_full[:, m, :],
                        grtl[:, h * hr:(h + 1) * hr, m * ck:(m + 1) * ck],
                        start=(m == 0), stop=(m == oc - 1))
                nc.vector.tensor_copy(
                    osb_rv[:, h * hr:(h + 1) * hr, off:off + ck], sph[:])

            # row-split the output store across both idle rings
            nc.sync.dma_start(out_d[:bc // 2, :], outsb[:bc // 2, :])
            nc.scalar.dma_start(out_d[bc // 2:, :], outsb[bc // 2:, :])

    nc.compile()
    return nc


_NC_CACHE = {}


def _get_nc():
    key = (B_FULL, C_IN, T_POOL, O_OUT, N_CORES)
    if key not in _NC_CACHE:
        _NC_CACHE[key] = build_kernel(*key)
    return _NC_CACHE[key]


def _run(features, W, bias, trace=False, tmpdir=None):
    from concourse.bass_utils import run_bass_kernel_spmd

    feats = np.ascontiguousarray(np.asarray(features, dtype=np.float32))
    w_np = np.ascontiguousarray(np.asarray(W, dtype=np.float32))
    bias_np = np.ascontiguousarray(
        np.asarray(bias, dtype=np.float32).reshape(1, O_OUT))
    bc = B_FULL // N_CORES

    nc = _get_nc()
    in_maps = [
        {"features": feats[r * bc:(r + 1) * bc], "w": w_np, "bias": bias_np}
        for r in range(N_CORES)
    ]
    kw = {"tmpdir": tmpdir} if tmpdir else {}
    res = run_bass_kernel_spmd(nc, in_maps, core_ids=list(range(N_CORES)),
                               trace=trace, **kw)
    out = np.concatenate([res.results[r]["out"] for r in range(N_CORES)], axis=0)
    return out, res.exec_time_ns


def kernel(features, W, bias):
    out, _ = _run(features, W, bias)
    return out


# revision 9
# speedup vs baseline: 1.1874x; 1.0868x over previous
"""Chunked bf16 kernel with per-chunk ncfw AllGathers (4 x 16 batches).

r3 design (evidence from the r1/r2 traces):
- the feature stream runs at the HBM cap (~357 GB/s) when nothing gates
  it: per-batch DMAs alternate the two HWDGE rings, the last 4 batches
  stream j-group-major from a dedicated 16-buf pool (no reduce-gating)
- ncfw mesh AllGather cost grows steeply with payload (16KB -> ~24us,
  32KB -> ~30-46us, 48KB -> ~97us), so use four 16KB AGs: the first
  three hide under the stream, only the last one (~24us) is exposed
- mid-chunk collective-adjacent work stays off the stream rings and off
  DVE: norm via ACT square/sqrt + gpsimd normalize, copies on ACT,
  agin/grt DMAs on gpsimd (SWDGE)
- tail: last chunk normalizes via ACT muls + DVE copies for the fastest
  possible trigger, all sim matmuls run during the final AG wait, one
  row-split output store at the end."""

import sys

if "/opt/trn_rl_repo" not in sys.path:
    sys.path.insert(0, "/opt/trn_rl_repo")

import numpy as np

B_FULL = 512
C_IN = 2048
T_POOL = 196
O_OUT = 512
N_CORES = 8

CKS = [32, 16, 16]       # batches per chunk; one AllGather per chunk
JSPLIT = 4               # trailing batches of the LAST chunk streamed j-major
JGROUPS = [4, 4, 4, 2, 2]  # j-group unit sizes for the j-major tail


def build_kernel(b_full, c_in, t_pool, o_out, n_cores, ft_bufs=5):
    import concourse.mybir as mybir
    import concourse.tile as tile
    from concourse import bacc
    from concourse.masks import make_identity

    f32 = mybir.dt.float32
    bf16 = mybir.dt.bfloat16
    AL = mybir.AluOpType
    AF = mybir.ActivationFunctionType
    X = mybir.AxisListType.X

    bc = b_full // n_cores
    nj = 16
    cks = list(CKS)
    nch = len(cks)
    offs = [sum(cks[:c]) for c in range(nch)]
    ckmax = max(cks)
    oc = o_out // 128
    nr = n_cores
    assert c_in == 128 * nj and sum(cks) == bc and o_out % 128 == 0

    nc = bacc.Bacc("TRN2", target_bir_lowering=False, debug=False,
                   enable_asserts=False, num_devices=n_cores)
    feat = nc.dram_tensor("features", [bc, c_in, t_pool], f32,
                          kind="ExternalInput").ap()
    w_in = nc.dram_tensor("w", [o_out, c_in], f32, kind="ExternalInput").ap()
    bias_in = nc.dram_tensor("bias", [1, o_out], f32, kind="ExternalInput").ap()
    out_d = nc.dram_tensor("out", [bc, b_full], f32, kind="ExternalOutput").ap()

    with tile.TileContext(nc) as tc:
        with (
            tc.tile_pool(name="const", bufs=1) as constp,
            tc.tile_pool(name="wload", bufs=1) as wlp,
            tc.tile_pool(name="wtp", bufs=1) as wtp,
            tc.tile_pool(name="featp", bufs=ft_bufs) as fp,
            tc.tile_pool(name="featl", bufs=len(JGROUPS)) as flp,
            tc.tile_pool(name="poolp", bufs=1) as lp,
            tc.tile_pool(name="normp", bufs=2) as np_,
            tc.tile_pool(name="postp", bufs=1) as pp,
            tc.tile_pool(name="psrot", bufs=2, space="PSUM") as psp,
            tc.tile_pool(name="psgps", bufs=2, space="PSUM") as psgp,
            tc.tile_pool(name="pssim", bufs=2, space="PSUM") as pssp,
            tc.tile_pool(name="dram", bufs=1, space="DRAM") as dp,
        ):
            # ---- constants ----
            identf = constp.tile([128, 128], f32, name="identf")
            make_identity(nc, identf)
            identb = constp.tile([ckmax, ckmax], bf16, name="identb")
            make_identity(nc, identb)
            ones = constp.tile([1, ckmax], bf16, name="ones")
            nc.vector.memset(ones, 1.0)
            bias_sb = constp.tile([1, o_out], f32, name="bias_sb")
            nc.sync.dma_start(bias_sb[:], bias_in[:])
            bias_t = constp.tile([1, o_out], bf16, name="bias_t")
            nc.scalar.mul(bias_t[:], bias_sb[:], float(t_pool))

            # ---- W^T in bf16 ----
            wl = []
            for l in range(oc):
                wli = wlp.tile([128, c_in], f32, name=f"wl{l}")
                eng = nc.sync if l % 2 == 0 else nc.scalar
                eng.dma_start(wli[:], w_in[l * 128:(l + 1) * 128, :])
                wl.append(wli)
            wt = []
            for j in range(nj):
                pswt = psp.tile([128, o_out], f32, name="pswt", tag="rot")
                for l in range(oc):
                    src = wl[l][:, :].rearrange("o (p j) -> o p j", j=nj)[:, :, j]
                    nc.tensor.transpose(pswt[:, l * 128:(l + 1) * 128],
                                        src, identf[:])
                wtj = wtp.tile([128, o_out], bf16, name=f"wt{j}")
                nc.scalar.copy(wtj[:], pswt[:])
                wt.append(wtj)

            gl_full = pp.tile([128, oc, bc], bf16, name="gl_full")
            outsb = pp.tile([bc, b_full], f32, name="outsb")
            glcs = [pp.tile([128, oc * cks[c]], bf16, name=f"glc{c}")
                    for c in range(nch)]
            agouts = []
            grts = []

            def pool_chunk(c):
                off, ck = offs[c], cks[c]
                p4 = lp.tile([128, ck, nj], bf16, name=f"p4_{c}")
                split = JSPLIT if c == nch - 1 else 0
                for i in range(ck - split):
                    b = off + i
                    ft = fp.tile([128, nj * t_pool], f32, name="ft")
                    src = feat[b:b + 1, :, :].rearrange(
                        "b (p j) t -> p (b j t)", j=nj)
                    dma_eng = nc.scalar if b % 2 == 0 else nc.sync
                    dma_eng.dma_start(ft[:], src)
                    with nc.allow_low_precision("pooled sums cast to bf16"):
                        nc.vector.reduce_sum(
                            p4[:, i, :],
                            ft[:].rearrange("p (j t) -> p j t", t=t_pool),
                            axis=X)
                # last chunk: stream the trailing batches j-group-major.
                # One DMA + ONE multi-batch reduce per j-group unit: the
                # reduce has 4x less DVE instruction overhead than per-batch
                # slices, so DVE keeps pace with arrival and the post-stream
                # drain is just the final (small) unit. Final units cover
                # only 2 j's so the last reduce+matmul chain is short.
                if split:
                    b0 = off + ck - split
                    src4 = feat[b0:b0 + split, :, :].rearrange(
                        "b (p j) t -> p b j t", j=nj)
                    joff = 0
                    for n, gj in enumerate(JGROUPS):
                        ftj = flp.tile([128, split, gj, t_pool], f32,
                                       name="ftl")
                        eng = nc.scalar if n % 2 == 0 else nc.sync
                        eng.dma_start(ftj[:], src4[:, :, joff:joff + gj, :])
                        with nc.allow_low_precision("pooled bf16"):
                            nc.vector.reduce_sum(
                                p4[:, ck - split:ck, joff:joff + gj],
                                ftj[:], axis=X)
                        joff += gj
                return p4

            def project(c, p4):
                ck = cks[c]
                gps = psgp.tile([ck, o_out], f32, name="gps", tag="gps")
                for j in range(nj):
                    nc.tensor.matmul(gps[:], p4[:, :, j], wt[j][:],
                                     start=(j == 0), stop=False)
                nc.tensor.matmul(gps[:], ones[:, :ck], bias_t[:],
                                 start=False, stop=True)
                return gps

            def start_ag(c, dma_eng):
                ck = cks[c]
                agin = dp.tile([128, oc * ck], bf16, name=f"agin{c}")
                agout = dp.tile([nr * 128, oc * ck], bf16, name=f"agout{c}",
                                addr_space="Shared")
                dma_eng.dma_start(agin[:], glcs[c][:])
                nc.gpsimd.collective_compute(
                    "AllGather", AL.bypass,
                    replica_groups=[list(range(n_cores))],
                    ins=[agin.opt()], outs=[agout.opt()],
                )
                agouts.append(agout)

            # ================= mid chunks =================
            for c in range(nch - 1):
                ck, off = cks[c], offs[c]
                p4 = pool_chunk(c)
                gps = project(c, p4)
                scr = np_.tile([ck, o_out], f32, name="scr")
                n2 = np_.tile([ck, 1], f32, name="n2")
                nc.scalar.activation(scr[:], gps[:], AF.Square,
                                     accum_out=n2[:])
                gsb = np_.tile([ck, o_out], f32, name="gsb")
                nc.scalar.copy(gsb[:], gps[:])
                nrm = np_.tile([ck, 1], f32, name="nrm")
                nc.scalar.sqrt(nrm[:], n2[:])
                gn = np_.tile([ck, o_out], bf16, name="gn")
                nc.gpsimd.normalize_recip(gn[:], gsb[:], nrm[:])
                glc_v = glcs[c][:].rearrange("p (m i) -> p m i", i=ck)
                for m in range(oc):
                    psg = psp.tile([128, ck], bf16, name="psg", tag="rot")
                    nc.tensor.transpose(psg[:], gn[:, m * 128:(m + 1) * 128],
                                        identb[:ck, :ck])
                    nc.scalar.copy(gl_full[:, m, off:off + ck], psg[:])
                    nc.scalar.copy(glc_v[:, m, :], psg[:])
                # collective + gather load stay on the SWDGE ring so the
                # HWDGE feature stream is never queued behind them
                start_ag(c, nc.gpsimd)
                grt = pp.tile([128, nr, oc * ck], bf16, name=f"grt{c}")
                nc.gpsimd.dma_start(
                    grt[:],
                    agouts[c][:, :].rearrange("(r p) f -> p r f", r=nr))
                grts.append(grt)

            # ================= last chunk =================
            cl = nch - 1
            ck, off = cks[cl], offs[cl]
            p4 = pool_chunk(cl)
            gps = project(cl, p4)
            scr1 = np_.tile([ck, o_out], f32, name="scr")
            n21 = np_.tile([ck, 1], f32, name="n2")
            nc.scalar.activation(scr1[:], gps[:], AF.Square, accum_out=n21[:])
            nrm1 = np_.tile([ck, 1], f32, name="nrm")
            nc.scalar.sqrt(nrm1[:], n21[:])
            rinv1 = pp.tile([ck, 1], f32, name="rinv1")
            nc.vector.reciprocal(rinv1[:], nrm1[:])
            gn1 = np_.tile([ck, o_out], bf16, name="gn")
            glc_v1 = glcs[cl][:].rearrange("p (m i) -> p m i", i=ck)
            # scale per 128-block so each transpose starts as soon as its
            # block is written; ALL copies on DVE so the ACT chain is just
            # square -> sqrt -> muls
            for m in range(oc):
                nc.scalar.mul(gn1[:, m * 128:(m + 1) * 128],
                              gps[:, m * 128:(m + 1) * 128], rinv1[:])
                psg = psp.tile([128, ck], bf16, name="psg", tag="rot")
                nc.tensor.transpose(psg[:], gn1[:, m * 128:(m + 1) * 128],
                                    identb[:ck, :ck])
                nc.vector.tensor_copy(glc_v1[:, m, :], psg[:])
                nc.vector.tensor_copy(gl_full[:, m, off:off + ck], psg[:])
            # trigger the final AG ASAP; sync ring is idle once the stream
            # is done
            start_ag(cl, nc.sync)

            # ---- sims for the mid chunks run during the final AG wait ----
            osb_rv = outsb[:, :].rearrange("b (r i) -> b r i", i=bc)
            for c in range(nch - 1):
                ck, off = cks[c], offs[c]
                simps = pssp.tile([bc, nr * ck], f32, name="simps",
                                  tag=f"sim{c % 2}")
                for m in range(oc):
                    nc.tensor.matmul(
                        simps[:], gl_full[:, m, :],
                        grts[c][:, :, m * ck:(m + 1) * ck],
                        start=(m == 0), stop=(m == oc - 1))
                nc.vector.tensor_copy(osb_rv[:, :, off:off + ck], simps[:])

            # ---- final chunk: split gather load, per-half sims ----
            grtl = pp.tile([128, nr, oc * ck], bf16, name=f"grt{cl}")
            agvl = agouts[cl][:, :].rearrange("(r p) f -> p r f", r=nr)
            hr = nr // 2
            nc.sync.dma_start(grtl[:, :hr, :], agvl[:, :hr, :])
            nc.scalar.dma_start(grtl[:, hr:, :], agvl[:, hr:, :])
            ck, off = cks[cl], offs[cl]
            for h in range(2):
                sph = pssp.tile([bc, hr * ck], f32, name=f"simh{h}",
                                tag=f"sim{h}")
                for m in range(oc):
                    nc.tensor.matmul(
                        sph[:], gl_full[:, m, :],
                        grtl[:, h * hr:(h + 1) * hr, m * ck:(m + 1) * ck],
                        start=(m == 0), stop=(m == oc - 1))
                nc.vector.tensor_copy(
                    osb_rv[:, h * hr:(h + 1) * hr, off:off + ck], sph[:])

            # row-split the output store across both idle rings
            nc.sync.dma_start(out_d[:bc // 2, :], outsb[:bc // 2, :])
            nc.scalar.dma_start(out_d[bc // 2:, :], outsb[bc // 2:, :])

    nc.compile()
    return nc


_NC_CACHE = {}


def _get_nc():
    key = (B_FULL, C_IN, T_POOL, O_OUT, N_CORES)
    if key not in _NC_CACHE:
        _NC_CACHE[key] = build_kernel(*key)
    return _NC_CACHE[key]


def _run(features, W, bias, trace=False, tmpdir=None):
    from concourse.bass_utils import run_bass_kernel_spmd

    feats = np.ascontiguousarray(np.asarray(features, dtype=np.float32))
    w_np = np.ascontiguousarray(np.asarray(W, dtype=np.float32))
    bias_np = np.ascontiguousarray(
        np.asarray(bias, dtype=np.float32).reshape(1, O_OUT))
    bc = B_FULL // N_CORES

    nc = _get_nc()
    in_maps = [
        {"features": feats[r * bc:(r + 1) * bc], "w": w_np, "bias": bias_np}
        for r in range(N_CORES)
    ]
    kw = {"tmpdir": tmpdir} if tmpdir else {}
    res = run_bass_kernel_spmd(nc, in_maps, core_ids=list(range(N_CORES)),
                               trace=trace, **kw)
    out = np.concatenate([res.results[r]["out"] for r in range(N_CORES)], axis=0)
    return out, res.exec_time_ns


def kernel(features, W, bias):
    out, _ = _run(features, W, bias)
    return out
